# revision 1
# baseline (speedup 1.0000x reference)
"""Trainium2 Bass kernel for per-cluster block-diagonal attention + MLP.

Reference computation (per batch b of 8):
    q,k,v = x @ W{q,k,v}.T + b{q,k,v}        x: [4096, 3]
    S     = q @ k.T / sqrt(3)                 masked to same-cluster pairs
    attn  = softmax(S)  (noise rows -> ctx = 0)
    ctx   = attn @ v
    out   = ctx @ Wo.T + bo
    y     = relu(out @ W1.T + b1) @ W2.T + b2
    return y[:, :1024]

Strategy (one batch per NeuronCore, 8 cores data-parallel):
  * Only the first 1024 queries are needed (output slice); keys span all 4096.
  * Scores S^T[j,i] = k_j . q_i are computed as x_j . (Wk^T q_i) + bk . q_i so
    the raw x is the stationary operand; q-side factors fold into a 4x4 host
    matrix G applied on device.
  * f16 hi/lo split: S = x_hi.q_hi + x_hi.q_lo + x_lo.q_hi (fp32-grade
    precision at f16 matmul speed).  The 128-row stationary holds blocks at
    32-aligned offsets: [x_hi|1], [x_hi|1], [x_lo], [onehot8(a)|onehot8(b)].
  * Cluster mask folded into the same matmul: labels+1 are split into base-8
    digits (a,b); 8-row one-hots of each digit on both sides contribute
    BIG per matching digit.  exp(scale*S + 2*BIG*scale*match - 2*BIG*scale)
    zeroes any pair that does not match in both digits.
  * Unnormalized ctx (v in hi/lo columns) and the denominator Z accumulate in
    one PSUM tile via a [128, 33] stationary per 128-key chunk.
  * Epilogue (out-proj + MLP on 1024 rows) runs in plain fp32.
"""

import numpy as np
import ml_dtypes
from contextlib import ExitStack

import concourse.bass as bass
import concourse.bacc as bacc
import concourse.tile as tile
from concourse import mybir
from concourse.bass_utils import run_bass_kernel_spmd

B, N, D, H, KQ, NCLUST = 8, 4096, 3, 256, 1024, 63
NCORES = 8
PJ = 128                 # keys per chunk
NCHUNK = N // PJ         # 32
MR = 96                  # mask block start row
ZROW = 64                # Z row within the ctx/Z accumulator
BIG = 1000.0
SCALE = float(1.0 / np.sqrt(np.float32(3.0)))

f32 = mybir.dt.float32
f16 = mybir.dt.float16
AF = mybir.ActivationFunctionType
OP = mybir.AluOpType

nph = np.float16

_CACHE = {}


def _build_bass(debug=False):
    nc = bacc.Bacc("TRN2", target_bir_lowering=False)
    if debug:
        d_dbgX = nc.dram_tensor("dbgX", [128, N], f32, kind="ExternalOutput")
        d_dbgR = nc.dram_tensor("dbgR", [128, KQ], f32, kind="ExternalOutput")
        d_dbgCZ = nc.dram_tensor("dbgCZ", [ZROW + 1, KQ], f32,
                                 kind="ExternalOutput")
        d_dbgE = nc.dram_tensor("dbgE", [128, KQ], f32, kind="ExternalOutput")
        d_dbgCTX = nc.dram_tensor("dbgCTX", [4, KQ], f32,
                                  kind="ExternalOutput")

    d_xhi = nc.dram_tensor("xhi4", [4, N], f16, kind="ExternalInput")
    d_xlo = nc.dram_tensor("xlo3", [3, N], f16, kind="ExternalInput")
    d_lab2 = nc.dram_tensor("labAB", [2, N], f16, kind="ExternalInput")
    d_labq = nc.dram_tensor("labq", [1, KQ], f32, kind="ExternalInput")
    d_labqpm = nc.dram_tensor("labqpm", [128, 8], f32, kind="ExternalInput")
    d_xq = nc.dram_tensor("xq", [4, KQ], f32, kind="ExternalInput")
    d_xpm = nc.dram_tensor("xpm", [128, NCHUNK * 6], f16, kind="ExternalInput")
    d_Gt = nc.dram_tensor("Gt", [4, 4], f32, kind="ExternalInput")
    d_wx = nc.dram_tensor("wx65", [ZROW + 1, 3], f32, kind="ExternalInput")
    d_bo = nc.dram_tensor("bo_c", [3, 1], f32, kind="ExternalInput")
    d_w1 = nc.dram_tensor("w1a", [4, H], f32, kind="ExternalInput")
    d_w2 = nc.dram_tensor("w2T", [H, 3], f32, kind="ExternalInput")
    d_b2 = nc.dram_tensor("b2c", [3, 1], f32, kind="ExternalInput")
    d_iota = nc.dram_tensor("iota16", [16, 1], f32, kind="ExternalInput")
    d_y = nc.dram_tensor("yT", [3, KQ], f32, kind="ExternalOutput")
    d_zs = nc.dram_tensor("zscratch", [1, KQ], f32, kind="Internal")
    d_rs = nc.dram_tensor("rscratch", [1, KQ], f32, kind="Internal")

    def bcast2x8(src_2row, width):
        # [2, width] -> [16, width]: row d*8+r reads src row d (partition
        # broadcast via 0-stride middle dim; DMA-only access pattern).
        # Row step is the source tensor's full row stride N, not `width`.
        return bass.AP(
            tensor=src_2row.tensor,
            offset=src_2row.offset,
            ap=[[N, 2], [0, 8], [1, width]],
        )

    with tile.TileContext(nc) as tc, ExitStack() as ctx:
        const = ctx.enter_context(tc.tile_pool(name="const", bufs=1))
        big = ctx.enter_context(tc.tile_pool(name="big", bufs=1))
        ebuf = ctx.enter_context(tc.tile_pool(name="ebuf", bufs=4))
        psS = ctx.enter_context(tc.tile_pool(name="psS", bufs=3, space="PSUM"))
        psCZ = ctx.enter_context(tc.tile_pool(name="psCZ", bufs=1, space="PSUM"))

        # ---- constants ----
        Gt_sb = const.tile([4, 4], f32)
        nc.sync.dma_start(Gt_sb, d_Gt[:, :])
        wx_sb = const.tile([ZROW + 1, 3], f32)
        nc.sync.dma_start(wx_sb, d_wx[:, :])
        bo_sb = const.tile([3, 1], f32)
        nc.sync.dma_start(bo_sb, d_bo[:, :])
        w1_sb = const.tile([4, H], f32)
        nc.sync.dma_start(w1_sb, d_w1[:, :])
        w2a_sb = const.tile([128, 3], f32)
        nc.sync.dma_start(w2a_sb, d_w2[0:128, :])
        w2b_sb = const.tile([128, 3], f32)
        nc.sync.dma_start(w2b_sb, d_w2[128:256, :])
        b2_sb = const.tile([3, 1], f32)
        nc.sync.dma_start(b2_sb, d_b2[:, :])
        iota_sb = const.tile([16, 1], f32)
        nc.sync.dma_start(iota_sb, d_iota[:, :])
        labq = const.tile([1, KQ], f32)
        nc.sync.dma_start(labq, d_labq[:, :])
        labqpm = const.tile([128, 8], f32)
        nc.sync.dma_start(labqpm, d_labqpm[:, :])
        nvpm = const.tile([128, 8], f32)
        nc.vector.tensor_scalar(out=nvpm, in0=labqpm, scalar1=-1.0,
                                scalar2=None, op0=OP.not_equal)
        xq_sb = const.tile([4, KQ], f32)
        nc.sync.dma_start(xq_sb, d_xq[:, :])
        exp_bias = const.tile([128, 1], f32)
        nc.vector.memset(exp_bias, -SCALE * 2.0 * BIG - 8.0)
        zero_bias = const.tile([128, 1], f32)
        nc.vector.memset(zero_bias, 0.0)

        # ---- stationary X [128, 4096] f16 ----
        # rows 0:3 x_hi | 3 ones | 32:35 x_hi | 35 ones | 64:67 x_lo
        # rows 96:104 onehot8(a_key) | 104:112 onehot8(b_key)
        X = big.tile([128, N], f16)
        nc.vector.memset(X, 0.0)
        nc.sync.dma_start(X[0:4, :], d_xhi[:, :])
        nc.sync.dma_start(X[32:36, :], d_xhi[:, :])
        nc.sync.dma_start(X[64:67, :], d_xlo[:, :])
        nc.sync.dma_start(X[4:7, :], d_xlo[:, :])
        nc.sync.dma_start(X[MR:MR + 16, :], bcast2x8(d_lab2[0:2, :], N))
        nc.vector.tensor_scalar(
            out=X[MR:MR + 16, :], in0=X[MR:MR + 16, :],
            scalar1=iota_sb, scalar2=None, op0=OP.is_equal,
        )

        # ---- moving R [128, 1024] f16 ----
        # rows 0:3 q_hi | 3 qb_hi | 32:35 q_lo | 35 qb_lo | 64:67 q_hi
        # rows 96:112 BIG*onehot8 of query digits
        R = big.tile([128, KQ], f16)
        nc.vector.memset(R, 0.0)
        ps_b = psS.tile([128, KQ], f32, tag="spsum")
        for hh in range(2):
            sl = slice(hh * 512, (hh + 1) * 512)
            nc.tensor.matmul(ps_b[0:4, sl], lhsT=Gt_sb, rhs=xq_sb[:, sl],
                             start=True, stop=True)
        nc.vector.tensor_copy(R[0:4, :], ps_b[0:4, :])          # hi (f16 cast)
        qlo4 = big.tile([4, KQ], f16)
        nc.vector.scalar_tensor_tensor(                          # lo = q - hi
            out=qlo4, in0=R[0:4, :], scalar=-1.0, in1=ps_b[0:4, :],
            op0=OP.mult, op1=OP.add,
        )
        nc.sync.dma_start(R[32:36, :], qlo4)
        nc.sync.dma_start(R[64:67, :], R[0:3, :])
        nc.sync.dma_start(R[MR:MR + 16, :], bcast2x8(d_lab2[0:2, 0:KQ], KQ))
        nc.vector.tensor_scalar(
            out=R[MR:MR + 16, :], in0=R[MR:MR + 16, :],
            scalar1=iota_sb, scalar2=BIG, op0=OP.is_equal, op1=OP.mult,
        )

        if debug:
            dbgXs = big.tile([128, N], f32)
            nc.scalar.activation(dbgXs, X, AF.Copy)
            nc.sync.dma_start(d_dbgX[:, :], dbgXs)
            dbgRs = big.tile([128, KQ], f32)
            nc.scalar.activation(dbgRs, R, AF.Copy)
            nc.sync.dma_start(d_dbgR[:, :], dbgRs)

        # ---- prebuild all 32 ctx/Z stationaries [128, 65] from host xpm ----
        VW = ZROW + 1
        xpm_sb = big.tile([128, NCHUNK * 6], f16)
        nc.sync.dma_start(xpm_sb, d_xpm[:, :])
        vcall = big.tile([128, NCHUNK * VW], f16)
        vc_view = vcall.rearrange("p (j c) -> p j c", c=VW)
        xp_view = xpm_sb.rearrange("p (j c) -> p j c", c=6)
        nc.vector.memset(vcall, 0.0)
        nc.vector.tensor_copy(vc_view[:, :, 0:3], xp_view[:, :, 0:3])   # x_hi
        nc.vector.tensor_copy(vc_view[:, :, 32:35], xp_view[:, :, 3:6])  # x_lo
        nc.vector.memset(vc_view[:, :, ZROW:ZROW + 1], 1.0)

        # ---- main loop over 32 key chunks, cz skewed one chunk behind ----
        cz = psCZ.tile([ZROW + 1, KQ], f32)
        SKEW = 2
        Es = [None] * NCHUNK
        for j in range(NCHUNK + SKEW):
            if j < NCHUNK:
                Xj = X[:, j * PJ:(j + 1) * PJ]
                ps_s = psS.tile([128, KQ], f32, tag="spsum", name=f"ps_s_{j}")
                for hh in range(2):
                    sl = slice(hh * 512, (hh + 1) * 512)
                    nc.tensor.matmul(ps_s[:, sl], lhsT=Xj, rhs=R[:, sl],
                                     start=True, stop=True)
                E = ebuf.tile([128, KQ], f16, tag="E", name=f"E_{j}")
                nc.scalar.activation(E, ps_s, AF.Exp, bias=exp_bias,
                                     scale=SCALE)
                Es[j] = E
                if debug and j == 0:
                    dbgEs = big.tile([128, KQ], f32)
                    nc.scalar.activation(dbgEs, E, AF.Copy)
                    nc.sync.dma_start(d_dbgE[:, :], dbgEs)
            if j >= SKEW:
                jj = j - SKEW
                for hh in range(2):
                    sl = slice(hh * 512, (hh + 1) * 512)
                    nc.tensor.matmul(cz[:, sl], lhsT=vc_view[:, jj, :],
                                     rhs=Es[jj][:, sl],
                                     start=(jj == 0), stop=(jj == NCHUNK - 1))

        # ---- epilogue: ctx = (num_hi+num_lo)/Z (0 for noise), MLP fp32 ----
        # reciprocal in [128, 8] layout (8 elems/lane instead of 1024):
        # zpm[p, t] = Z[t*128 + p]
        zrow_sb = big.tile([1, KQ], f32)
        nc.scalar.activation(zrow_sb, cz[ZROW:ZROW + 1, :], AF.Copy)
        # bounce through DRAM to reshape [1,1024] <-> [128,8] across partitions
        nc.sync.dma_start(d_zs[:, :], zrow_sb)
        zpm = big.tile([128, 8], f32)
        zsrc = bass.AP(tensor=d_zs[:, :].tensor, offset=0,
                       ap=[[1, 128], [128, 8]])
        nc.sync.dma_start(zpm, zsrc)
        rzpm = big.tile([128, 8], f32)
        nc.vector.reciprocal(rzpm, zpm)
        nc.vector.tensor_tensor(out=rzpm, in0=rzpm, in1=nvpm, op=OP.mult)
        rdst = bass.AP(tensor=d_rs[:, :].tensor, offset=0,
                       ap=[[1, 128], [128, 8]])
        nc.sync.dma_start(rdst, rzpm)
        rZ = big.tile([1, KQ], f32)
        nc.sync.dma_start(rZ, d_rs[:, :])
        rzb = big.tile([36, KQ], f32)
        nc.gpsimd.partition_broadcast(rzb, rZ)
        val1 = big.tile([1, KQ], f32)
        nc.vector.tensor_scalar(out=val1, in0=labq, scalar1=-1.0,
                                scalar2=None, op0=OP.not_equal)

        ctxTa = big.tile([ZROW + 1, KQ], f32)
        nc.vector.memset(ctxTa, 0.0)
        nc.vector.tensor_tensor(out=ctxTa[0:3, :], in0=cz[0:3, :],
                                in1=rzb[0:3, :], op=OP.mult)
        nc.vector.tensor_tensor(out=ctxTa[32:35, :], in0=cz[32:35, :],
                                in1=rzb[32:35, :], op=OP.mult)
        nc.sync.dma_start(ctxTa[ZROW:ZROW + 1, :], val1)

        if debug:
            dbgCZs = big.tile([ZROW + 1, KQ], f32)
            nc.vector.tensor_copy(dbgCZs, cz)
            nc.sync.dma_start(d_dbgCZ[:, :], dbgCZs)
            nc.sync.dma_start(d_dbgCTX[:, :], ctxTa)
        ps_o = psS.tile([3, KQ], f32, tag="spsum")
        for hh in range(2):
            sl = slice(hh * 512, (hh + 1) * 512)
            nc.tensor.matmul(ps_o[:, sl], lhsT=wx_sb, rhs=ctxTa[:, sl],
                             start=True, stop=True)
        outTa = big.tile([4, KQ], f32)
        nc.vector.memset(outTa, 1.0)
        nc.scalar.activation(outTa[0:3, :], ps_o[0:3, :], AF.Identity,
                             bias=bo_sb, scale=1.0)

        hts = []
        for half in range(2):
            ps_h = psS.tile([128, KQ], f32, tag="spsum", name=f"ps_h_{half}")
            wsl = w1_sb[:, half * 128:(half + 1) * 128]
            for hh in range(2):
                sl = slice(hh * 512, (hh + 1) * 512)
                nc.tensor.matmul(ps_h[:, sl], lhsT=wsl, rhs=outTa[:, sl],
                                 start=True, stop=True)
            hT = big.tile([128, KQ], f32, name=f"hT_{half}")
            nc.scalar.activation(hT, ps_h, AF.Relu, bias=zero_bias[0:128])
            hts.append(hT)

        ps_y = psS.tile([3, KQ], f32, tag="spsum")
        for half, w2c in enumerate([w2a_sb, w2b_sb]):
            for hh in range(2):
                sl = slice(hh * 512, (hh + 1) * 512)
                nc.tensor.matmul(ps_y[:, sl], lhsT=w2c, rhs=hts[half][:, sl],
                                 start=(half == 0), stop=(half == 1))
        yT = big.tile([3, KQ], f32)
        nc.scalar.activation(yT, ps_y, AF.Identity, bias=b2_sb, scale=1.0)
        nc.sync.dma_start(d_y[:, :], yT)

    nc.finalize()
    return nc


def _hi_lo(a):
    hi = a.astype(nph)
    lo = (a.astype(np.float32) - hi.astype(np.float32)).astype(nph)
    return hi, lo


def _prep_consts(Wq, bq, Wk, bk, Wv, bv, Wo, bo, W1, b1, W2, b2):
    Wq, bq, Wk, bk = [np.asarray(a, np.float32) for a in (Wq, bq, Wk, bk)]
    Wv, bv, Wo, bo = [np.asarray(a, np.float32) for a in (Wv, bv, Wo, bo)]
    W1, b1, W2, b2 = [np.asarray(a, np.float32) for a in (W1, b1, W2, b2)]

    G = np.zeros((4, 4), np.float32)
    G[0:3, 0:3] = Wk.T @ Wq
    G[0:3, 3] = Wk.T @ bq
    G[3, 0:3] = bk @ Wq
    G[3, 3] = bk @ bq
    Gt = np.ascontiguousarray(G.T)


    WoWv = (Wo.astype(np.float64) @ Wv.astype(np.float64)).astype(np.float32)
    wx65 = np.zeros((65, 3), np.float32)
    wx65[0:3, :] = WoWv.T
    wx65[32:35, :] = WoWv.T
    wx65[64, :] = Wo @ bv
    bo_c = np.ascontiguousarray(bo[:, None]).astype(np.float32)
    w1a = np.concatenate([W1.T, b1[None, :]], axis=0).astype(np.float32)
    w2T = np.ascontiguousarray(W2.T).astype(np.float32)
    b2c = np.ascontiguousarray(b2[:, None]).astype(np.float32)
    iota16 = np.concatenate([np.arange(8), np.arange(8)]).astype(np.float32)[:, None]
    iota16 = np.ascontiguousarray(iota16)
    return dict(Gt=Gt, wx65=wx65, bo_c=bo_c, w1a=w1a, w2T=w2T, b2c=b2c,
                iota16=iota16)


def kernel(x, labels, Wq, bq, Wk, bk, Wv, bv, Wo, bo, W1, b1, W2, b2,
           _trace=False):
    x = np.asarray(x, np.float32)
    labi = np.asarray(labels).astype(np.int64)

    consts = _prep_consts(Wq, bq, Wk, bk, Wv, bv, Wo, bo, W1, b1, W2, b2)

    if "nc" not in _CACHE:
        _CACHE["nc"] = _build_bass()
    nc = _CACHE["nc"]

    ones_row = np.ones((1, N), np.float32)
    in_maps = []
    for b in range(B):
        xT = x[b].T                                   # [3, 4096]
        xh, xl = _hi_lo(xT)
        xhi4 = np.concatenate([xh, ones_row.astype(nph)], axis=0)
        # partition-major x hi/lo for the ctx/Z stationaries:
        # xpm[p, j*6+c] = hi(x)[j*128+p, c], +3 for lo
        xpm = np.zeros((128, NCHUNK * 6), nph)
        xpm3 = xh.T.reshape(NCHUNK, 128, 3)
        xpl3 = xl.T.reshape(NCHUNK, 128, 3)
        for c in range(3):
            xpm[:, c::6] = xpm3[:, :, c].T
            xpm[:, 3 + c::6] = xpl3[:, :, c].T
        v = labi[b] + 1                               # 0..63
        labAB = np.stack([v >> 3, v & 7]).astype(nph)
        m = {
            "xhi4": np.ascontiguousarray(xhi4),
            "xlo3": np.ascontiguousarray(xl),
            "labAB": np.ascontiguousarray(labAB),
            "labq": np.ascontiguousarray(
                labi[b][None, :KQ].astype(np.float32)),
            "labqpm": np.ascontiguousarray(
                labi[b][:KQ].reshape(8, 128).T.astype(np.float32)),
            "xq": np.ascontiguousarray(
                np.concatenate([xT[:, :KQ], ones_row[:, :KQ]],
                               axis=0).astype(np.float32)),
            "xpm": np.ascontiguousarray(xpm),
        }
        m.update(consts)
        in_maps.append(m)

    res = run_bass_kernel_spmd(nc, in_maps, core_ids=list(range(NCORES)),
                               trace=_trace)
    y = np.stack([np.asarray(res.results[b]["yT"]).T for b in range(B)])
    y = np.ascontiguousarray(y, np.float32)
    if _trace:
        _CACHE["last_exec_time_ns"] = res.exec_time_ns
        _CACHE["last_results"] = res
    return y

